# revision 1
# baseline (speedup 1.0000x reference)
"""Trainium2 Bass kernel for nn_ConvolutionAttention.

Reference computation (per batch element b of B=8):
  x1 = features1[b] as [C=256, 32, 32];  x2 = features2[b] likewise
  q = pw(bn(dw3x3(x1)));  k = pw(bn(dw3x3(x2)));  v same as k w/ own weights
  per head h (8 heads, dh=64): attn = softmax(q_h k_h^T / 8);  o_h = attn v_h
  out[b] = concat_h(o_h) @ ffn_w.T + ffn_b      -> [1024, 256]

Sharding: pure data-parallel over batch; core i computes batch element i.

Per-core layout strategy (all matmuls in f32r = TF32):
  - host pre-transposes/pads features to [2, 128, 34*34]; BN + biases folded
    into dw-diag matrices / pw bias vectors on host.
  - depthwise conv = 9 shifted diagonal matmuls accumulating in PSUM.
  - q, k pointwise conv in [oc, hw] layout; v pointwise computed transposed
    [hw, oc] so attention needs no on-chip transposes.
  - scores computed transposed s_T[j, i] = k_h^T q_h (both operands natural);
    exp on ACT straight from PSUM (scores in [-0.12, 0.12] so no max-sub);
    attn@v via lhsT = [v_h^T | ones] (M=65) giving the softmax denominator in
    out row 64 for free; normalize via reciprocal + rank-1 PE broadcast.
  - ffn produces [hw, C] directly (per-head K=64 chunks).
"""

import numpy as np

import concourse.bass as bass
import concourse.bacc as bacc
import concourse.tile as tile
from concourse import mybir
from concourse.bass_utils import run_bass_kernel_spmd

F32 = mybir.dt.float32
F32R = mybir.dt.float32r
BF16 = mybir.dt.bfloat16

B, C, HWN, H, W = 8, 256, 1024, 32, 32
HEADS, DH, OC = 8, 64, 512
SCALE = DH ** -0.5
EPS = 1e-5
PAD = 34 * 34  # 1156

_CACHE = {}


# ----------------------------------------------------------------- device code

def _emit(nc, tc):
    # ---- DRAM I/O ----
    xq = nc.dram_tensor("xq", [2, 128, PAD], F32R, kind="ExternalInput").ap()
    xkv = nc.dram_tensor("xkv", [2, 128, PAD], F32R, kind="ExternalInput").ap()
    eye = nc.dram_tensor("eye", [128, 128], F32R, kind="ExternalInput").ap()
    dwt = nc.dram_tensor("dwt", [128, 54], F32R, kind="ExternalInput").ap()
    wq = nc.dram_tensor("wq", [2, 128, 512], F32R, kind="ExternalInput").ap()
    wk = nc.dram_tensor("wk", [2, 128, 512], F32R, kind="ExternalInput").ap()
    wv = nc.dram_tensor("wv", [2, 128, 512], F32R, kind="ExternalInput").ap()
    qk_bias = nc.dram_tensor("qk_bias", [128, 8], F32, kind="ExternalInput").ap()
    vbias = nc.dram_tensor("vbias", [1, 512], F32R, kind="ExternalInput").ap()
    vt_ones = nc.dram_tensor("vt_ones", [128, 8, 1], F32R, kind="ExternalInput").ap()
    ones_all = nc.dram_tensor("ones_all", [128, 128], F32R, kind="ExternalInput").ap()
    # ffn_w.T in chunks: [4, 128, 256]
    ffnw = nc.dram_tensor("ffnw", [4, 128, 256], F32R, kind="ExternalInput").ap()
    ffnb = nc.dram_tensor("ffnb", [1, 256], F32R, kind="ExternalInput").ap()
    out = nc.dram_tensor("out", [HWN, C], F32, kind="ExternalOutput").ap()

    mm = nc.tensor.matmul

    with nc.allow_low_precision(reason="f32r matmul pipeline"):
        _emit_body(nc, tc, locals())


def _emit_body(nc, tc, d):
    mm = nc.tensor.matmul
    xq, xkv, eye, dwt, qk_bias, vbias, vt_ones, ones_all, ffnw, ffnb, out = (
        d["xq"], d["xkv"], d["eye"], d["dwt"], d["qk_bias"], d["vbias"],
        d["vt_ones"], d["ones_all"], d["ffnw"], d["ffnb"], d["out"])
    wmap = {"q": d["wq"], "k": d["wk"], "v": d["wv"]}

    with tc.tile_pool(name="const", bufs=1) as const:
        # persistent weights / biases
        w_sb = {p: [const.tile([128, 512], F32R, tag=f"w{p}{kc}", name=f"w{p}{kc}") for kc in range(2)]
                for p in ("q", "k", "v")}
        ffnw_sb = [const.tile([128, 256], F32R, tag=f"ffnw{h}", name=f"ffnw{h}") for h in range(4)]
        for h in range(4):
            nc.sync.dma_start(ffnw_sb[h][:], ffnw[h])
        qkb_sb = const.tile([128, 8], F32, tag="qkb", name="qkb")
        nc.sync.dma_start(qkb_sb[:], qk_bias)
        vbias_sb = const.tile([1, 512], F32R, tag="vbias", name="vbiassb")
        nc.sync.dma_start(vbias_sb[:], vbias)
        ffnb_sb = const.tile([1, 256], F32R, tag="ffnb", name="ffnbsb")
        nc.sync.dma_start(ffnb_sb[:], ffnb)
        ones_sb = const.tile([128, 128], F32R, tag="ones", name="onessb")
        nc.sync.dma_start(ones_sb[:], ones_all)
        ones_f32 = const.tile([1, 64], F32, tag="ones32", name="ones_f32")
        nc.sync.dma_start(ones_f32[:], ones_all[0:1, 0:64].bitcast(F32))

        # persistent activations
        q_sb = [const.tile([128, HWN], F32R, tag=f"qsb{i}", name=f"qsb{i}") for i in range(4)]
        k_sb = [const.tile([128, HWN], F32R, tag=f"ksb{i}", name=f"ksb{i}") for i in range(4)]
        vt_sb = [const.tile([128, 8 * 66], F32R, tag=f"vt{i}", name=f"vt{i}") for i in range(8)]
        ot_sb = [const.tile([128, HWN], F32R, tag=f"ot{i}", name=f"ot{i}") for i in range(4)]

        # ---------------- phase 1: convolutions ----------------
        with tc.tile_pool(name="p1", bufs=1) as p1, \
             tc.tile_pool(name="psdw", bufs=2, space="PSUM") as psdw, \
             tc.tile_pool(name="pspw", bufs=2, space="PSUM") as pspw:
            eye_sb = p1.tile([128, 128], F32R, tag="eye", name="eye_sb")
            nc.sync.dma_start(eye_sb[:], eye)
            dwt_sb = p1.tile([128, 54], F32R, tag="dwt", name="dwt_sb")
            nc.sync.dma_start(dwt_sb[:], dwt)
            x_sb = {}
            for nm, src in (("q", xq), ("kv", xkv)):
                for blk in range(2):
                    t = p1.tile([128, PAD], F32R, tag=f"x{nm}{blk}", name=f"x{nm}{blk}")
                    nc.sync.dma_start(t[:], src[blk])
                    x_sb[nm, blk] = t
            dwd_sb = {}
            for ci, p in enumerate(("q", "k", "v")):
                for blk in range(2):
                    t = p1.tile([128, 9 * 128], F32R, tag=f"dw{p}{blk}", name=f"dwt{p}{blk}")
                    i0 = ci * 18 + blk * 9
                    e3 = eye_sb[:].rearrange("p (a c) -> p a c", a=1)
                    w3 = dwt_sb[:, i0:i0 + 9].rearrange("p (a c) -> p a c", c=1)
                    e3b, w3b = bass.broadcast_tensor_aps(e3, w3)
                    nc.vector.tensor_tensor(
                        t[:].rearrange("p (a c) -> p a c", c=128), e3b, w3b,
                        op=mybir.AluOpType.mult)
                    dwd_sb[p, blk] = t
            # weight loads after activations (off the critical startup path)
            for p in ("q", "k", "v"):
                for kc in range(2):
                    nc.sync.dma_start(w_sb[p][kc][:], wmap[p][kc])

            # depthwise 3x3 via 9 diagonal matmuls
            y_sb = {}
            cpy_eng = [nc.scalar, nc.vector]
            for ci, (p, xin) in enumerate((("q", "q"), ("k", "kv"), ("v", "kv"))):
                for blk in range(2):
                    ps = psdw.tile([128, HWN], F32, tag="dw", name="psdw")
                    xv = x_sb[xin, blk][:].rearrange("p (r c) -> p r c", c=34)
                    for tap in range(9):
                        di, dj = tap // 3, tap % 3
                        lhsT = dwd_sb[p, blk][:, tap * 128:(tap + 1) * 128]
                        for hf in range(2):
                            rhs = xv[:, di + hf * 16: di + hf * 16 + 16, dj: dj + 32]
                            mm(ps[:, hf * 512:(hf + 1) * 512], lhsT, rhs,
                               start=(tap == 0), stop=(tap == 8))
                    y = p1.tile([128, HWN], F32R, tag=f"y{p}{blk}", name=f"y{p}{blk}")
                    nc.vector.tensor_copy(y[:], ps[:])
                    y_sb[p, blk] = y

            # pointwise q, k in [oc, hw] layout (+bias via ACT)
            for ci, p in enumerate(("q", "k")):
                dest = q_sb if p == "q" else k_sb
                for mb in range(4):
                    ps = pspw.tile([128, HWN], F32, tag="pw", name="pspw")
                    for kc in range(2):
                        for hf in range(2):
                            mm(ps[:, hf * 512:(hf + 1) * 512],
                               w_sb[p][kc][:, mb * 128:(mb + 1) * 128],
                               y_sb[p, kc][:, hf * 512:(hf + 1) * 512],
                               start=(kc == 0), stop=(kc == 1))
                    nc.scalar.activation(
                        dest[mb][:], ps[:], mybir.ActivationFunctionType.Identity,
                        bias=qkb_sb[:, ci * 4 + mb: ci * 4 + mb + 1])

            # pointwise v, transposed: vt[hw, oc] (+bias via K=1 ones matmul)
            for mb in range(8):
                ps = pspw.tile([128, 512], F32, tag="pw", name="psvt")
                for kc in range(2):
                    mm(ps[:], y_sb["v", kc][:, mb * 128:(mb + 1) * 128],
                       w_sb["v"][kc][:], start=(kc == 0), stop=False)
                mm(ps[:], ones_sb[0:1, 0:128], vbias_sb[0:1, :],
                   start=False, stop=True)
                vtv = vt_sb[mb][:].rearrange("p (h c) -> p h c", c=66)
                nc.vector.tensor_copy(vtv[:, :, 0:64], ps[:])
                nc.sync.dma_start(vtv[:, :, 64:65], vt_ones)

        # ---------------- phase 2: attention ----------------
        with tc.tile_pool(name="p2", bufs=4) as p2, \
             tc.tile_pool(name="pss", bufs=2, space="PSUM") as pss, \
             tc.tile_pool(name="pso", bufs=1, space="PSUM") as pso:
            for pair in range(4):
                hA, hB = 2 * pair, 2 * pair + 1
                ops = {hA: pso.tile([65, HWN], F32, tag="oaccA", name="oaccA"),
                       hB: pso.tile([65, HWN], F32, tag="oaccB", name="oaccB")}
                e_q = []  # software pipeline: emit scores(jb+1) before av(jb)
                for jb in range(9):
                    if jb < 8:
                        e_t = {}
                        for h, pb in ((hA, 0), (hB, 64)):
                            sp = pss.tile([128, HWN], F32, tag="s", name="sp")
                            for hf in range(2):
                                mm(sp[:, hf * 512:(hf + 1) * 512],
                                   k_sb[pair][pb:pb + 64, jb * 128:(jb + 1) * 128],
                                   q_sb[pair][pb:pb + 64, hf * 512:(hf + 1) * 512],
                                   start=True, stop=True)
                            e = p2.tile([128, HWN], F32R, tag="e", name="e")
                            nc.scalar.activation(e[:], sp[:],
                                                 mybir.ActivationFunctionType.Exp,
                                                 scale=SCALE)
                            e_t[h] = e
                        e_q.append(e_t)
                    if jb >= 1:
                        e_t = e_q[jb - 1]
                        for h in (hA, hB):
                            for hf in range(2):
                                mm(ops[h][:, hf * 512:(hf + 1) * 512],
                                   vt_sb[jb - 1][:, 66 * h: 66 * h + 65],
                                   e_t[h][:, hf * 512:(hf + 1) * 512],
                                   start=(jb == 1), stop=(jb == 8))
                # normalize: o[d, i] / colsum[i]
                for h in (hA, hB):
                    o_un = p2.tile([65, HWN], F32, tag="oun", name="o_un", bufs=2)
                    nc.vector.tensor_copy(o_un[:], ops[h][:])
                    # reshape colsum row across 64 partitions for a cheap recip
                    csp = p2.tile([64, 16], F32, tag="csp", name="csp", bufs=2)
                    nc.sync.dma_start(
                        csp[:], o_un[64:65, :].rearrange("p (a b) -> p a b", b=16))
                    csr = p2.tile([64, 16], F32, tag="csr", name="csr", bufs=2)
                    nc.vector.reciprocal(csr[:], csp[:])
                    rrow = p2.tile([1, HWN], F32, tag="rrow", name="rrow", bufs=2)
                    nc.sync.dma_start(
                        rrow[:].rearrange("p (a b) -> p a b", b=16), csr[:])
                    bc = pso.tile([64, HWN], F32, tag=("oaccA" if h == hA else "oaccB"), name="bc")
                    for hf in range(2):
                        mm(bc[:, hf * 512:(hf + 1) * 512],
                           ones_f32[0:1, :],
                           rrow[0:1, hf * 512:(hf + 1) * 512],
                           start=True, stop=True)
                    otd = ot_sb[h // 2][(h % 2) * 64:(h % 2) * 64 + 64, :]
                    nc.vector.tensor_mul(otd, o_un[0:64, :], bc[:])

        # ---------------- phase 3: ffn ----------------
        with tc.tile_pool(name="p3", bufs=3) as p3, \
             tc.tile_pool(name="psf", bufs=2, space="PSUM") as psf:
            for nb in range(8):
                ps = psf.tile([128, 256], F32, tag="f", name="psf")
                for kc in range(4):
                    mm(ps[:], ot_sb[kc][:, nb * 128:(nb + 1) * 128], ffnw_sb[kc][:],
                       start=(kc == 0), stop=False)
                mm(ps[:], ones_sb[0:1, 0:128], ffnb_sb[0:1, :],
                   start=False, stop=True)
                fo = p3.tile([128, 256], F32, tag="fin", name="fin")
                nc.vector.tensor_copy(fo[:], ps[:])
                nc.sync.dma_start(out[nb * 128:(nb + 1) * 128, :], fo[:])


def _build():
    nc = bacc.Bacc("TRN2", target_bir_lowering=False, debug=False)
    with tile.TileContext(nc) as tc:
        _emit(nc, tc)
    nc.compile()
    return nc


# ----------------------------------------------------------------- host code

def _host_shared(inputs):
    g = lambda n: np.asarray(inputs[n], dtype=np.float32)
    d = {}
    dw_effs = []
    qk_bias_cols = []
    for ci, p in enumerate(("q", "k", "v")):
        a = g(f"{p}_bn_g") / np.sqrt(g(f"{p}_bn_v") + EPS)          # [256]
        dw_eff = g(f"{p}_dw_w")[:, 0] * a[:, None, None]            # [256,3,3]
        beta = a * g(f"{p}_dw_b") + g(f"{p}_bn_b") - a * g(f"{p}_bn_m")
        pw = g(f"{p}_pw_w")[:, :, 0, 0]                             # [512,256]
        bias = g(f"{p}_pw_b") + pw @ beta                           # [512]
        dw_effs.append(dw_eff)
        d[f"w{p}"] = np.ascontiguousarray(pw.T.reshape(2, 128, 512))
        if p == "v":
            d["vbias"] = bias.reshape(1, 512).copy()
        else:
            qk_bias_cols.append(bias)
    qkb = np.zeros((128, 8), np.float32)
    for ci in range(2):
        for mb in range(4):
            qkb[:, ci * 4 + mb] = qk_bias_cols[ci][mb * 128:(mb + 1) * 128]
    d["qk_bias"] = qkb
    # [3,2,9,128,128] -> [3,2,128,9*128]
    d["eye"] = np.eye(128, dtype=np.float32)
    dwt = np.zeros((128, 54), np.float32)
    for ci in range(3):
        for blk in range(2):
            for t in range(9):
                dwt[:, ci * 18 + blk * 9 + t] = dw_effs[ci][blk * 128:(blk + 1) * 128, t // 3, t % 3]
    d["dwt"] = dwt
    d["vt_ones"] = np.ones((128, 8, 1), np.float32)
    d["ones_all"] = np.ones((128, 128), np.float32)
    d["ffnw"] = np.ascontiguousarray(
        g("ffn_w").T.reshape(4, 128, 256))
    d["ffnb"] = g("ffn_b").reshape(1, 256).copy()
    return d


def _host_x(feat):
    # [1024, 256] -> padded transposed [2, 128, 34*34]
    xt = np.ascontiguousarray(feat.T).reshape(2, 128, 32, 32)
    xp = np.zeros((2, 128, 34, 34), np.float32)
    xp[:, :, 1:33, 1:33] = xt
    return xp.reshape(2, 128, PAD)


def make_in_maps(inputs):
    shared = _host_shared(inputs)
    f1 = np.asarray(inputs["features1"], dtype=np.float32)
    f2 = np.asarray(inputs["features2"], dtype=np.float32)
    maps = []
    for b in range(B):
        m = dict(shared)
        m["xq"] = _host_x(f1[b])
        m["xkv"] = _host_x(f2[b])
        maps.append(m)
    return maps


def get_nc():
    if "nc" not in _CACHE:
        _CACHE["nc"] = _build()
    return _CACHE["nc"]


def kernel(**inputs):
    nc = get_nc()
    in_maps = make_in_maps(inputs)
    res = run_bass_kernel_spmd(nc, in_maps, list(range(B)))
    return np.stack([res.results[i]["out"] for i in range(B)]).astype(np.float32)



# revision 12
# speedup vs baseline: 1.0823x; 1.0823x over previous
"""Trainium2 Bass kernel for nn_ConvolutionAttention (v2: bf16 pipeline).

Reference computation (per batch element b of B=8):
  x1 = features1[b] as [C=256, 32, 32];  x2 = features2[b] likewise
  q = pw(bn(dw3x3(x1)));  k = pw(bn(dw3x3(x2)));  v same as k w/ own weights
  per head h (8 heads, dh=64): attn = softmax(q_h k_h^T / 8);  o_h = attn v_h
  out[b] = concat_h(o_h) @ ffn_w.T + ffn_b      -> [1024, 256]

Sharding: pure data-parallel over batch; core i computes batch element i.

v2 design notes (vs the f32r baseline):
  - all matmul operands bf16: f32r streams 2 cycles/col on HW (fp32_mode=HIGH),
    bf16 streams 1 col/cycle -> ~2x on every matmul.
  - per-head attention (not head-pairs), software-pipelined against the
    convolutions so ACT-engine EXP (the 73us serial floor) starts ~20us in.
  - k pointwise bias kept (free via tensor_scalar on DVE); v pointwise bias
    folded through ffn_w into ffn_b on host (exact); ffn bias applied as a
    DVE tensor_tensor add with a host-replicated [128,256] tile (kills the
    K=1 bias matmuls).
  - softmax normalize: reciprocal straight off the PSUM denominator row
    ([1,1024] DVE op), rank-1 ones-matmul broadcast, DVE multiply. No SBUF
    DMA round trips.
  - ffn for nb-blocks 0..3 kc 0..2 pre-issued before the last head's av
    completes (PSUM banks freed by closing the A accumulator pool early).
"""

import numpy as np

import concourse.bass as bass
import concourse.bacc as bacc
import concourse.tile as tile
from concourse import mybir
from concourse.bass_utils import run_bass_kernel_spmd

F32 = mybir.dt.float32
BF16 = mybir.dt.bfloat16

B, C, HWN, H, W = 8, 256, 1024, 32, 32
HEADS, DH, OC = 8, 64, 512
SCALE = DH ** -0.5
EPS = 1e-5
PAD = 34 * 34  # 1156

_CACHE = {}


# ----------------------------------------------------------------- device code

def _emit(nc, tc):
    # ---- DRAM I/O ----
    xq = nc.dram_tensor("xq", [2, 128, PAD], BF16, kind="ExternalInput").ap()
    xkv = nc.dram_tensor("xkv", [2, 128, PAD], BF16, kind="ExternalInput").ap()
    eye = nc.dram_tensor("eye", [128, 128], BF16, kind="ExternalInput").ap()
    dwt = nc.dram_tensor("dwt", [128, 54], BF16, kind="ExternalInput").ap()
    wq = nc.dram_tensor("wq", [2, 128, 512], BF16, kind="ExternalInput").ap()
    wk = nc.dram_tensor("wk", [2, 128, 512], BF16, kind="ExternalInput").ap()
    wv = nc.dram_tensor("wv", [2, 128, 512], BF16, kind="ExternalInput").ap()
    qk_bias = nc.dram_tensor("qk_bias", [128, 8], F32, kind="ExternalInput").ap()
    vt_ones = nc.dram_tensor("vt_ones", [128, 8, 1], BF16, kind="ExternalInput").ap()
    ones_bc = nc.dram_tensor("ones_bc", [1, 64], BF16, kind="ExternalInput").ap()
    ffnw = nc.dram_tensor("ffnw", [4, 128, 256], BF16, kind="ExternalInput").ap()
    ffnb = nc.dram_tensor("ffnb", [128, 256], F32, kind="ExternalInput").ap()
    out = nc.dram_tensor("out", [HWN, C], F32, kind="ExternalOutput").ap()

    with nc.allow_low_precision(reason="bf16 matmul pipeline"):
        _emit_body(nc, tc, dict(xq=xq, xkv=xkv, eye=eye, dwt=dwt, wq=wq, wk=wk,
                                wv=wv, qk_bias=qk_bias, vt_ones=vt_ones,
                                ones_bc=ones_bc, ffnw=ffnw, ffnb=ffnb, out=out))


def _emit_body(nc, tc, d):
    mm = nc.tensor.matmul
    wmap = {"q": d["wq"], "k": d["wk"], "v": d["wv"]}

    with tc.tile_pool(name="const", bufs=1) as const:
        # small consts first (cheap DMAs, needed early)
        eye_sb = const.tile([128, 128], BF16, tag="eye", name="eye_sb")
        nc.sync.dma_start(eye_sb[:], d["eye"])
        dwt_sb = const.tile([128, 54], BF16, tag="dwt", name="dwt_sb")
        nc.sync.dma_start(dwt_sb[:], d["dwt"])
        # activations: xq first (critical path), then xkv
        x_sb = {}
        for nm, src in (("q", d["xq"]), ("kv", d["xkv"])):
            for blk in range(2):
                t = const.tile([128, PAD], BF16, tag=f"x{nm}{blk}", name=f"x{nm}{blk}")
                nc.sync.dma_start(t[:], src[blk])
                x_sb[nm, blk] = t
        # weights needed from ~10us on
        w_sb = {p: [const.tile([128, 512], BF16, tag=f"w{p}{kc}", name=f"w{p}{kc}")
                    for kc in range(2)] for p in ("q", "k", "v")}
        for p in ("q", "k", "v"):
            for kc in range(2):
                nc.sync.dma_start(w_sb[p][kc][:], wmap[p][kc])
        qkb_sb = const.tile([128, 8], F32, tag="qkb", name="qkb")
        nc.sync.dma_start(qkb_sb[:], d["qk_bias"])
        ones_bc_sb = const.tile([1, 64], BF16, tag="onesbc", name="ones_bc_sb")
        nc.sync.dma_start(ones_bc_sb[:], d["ones_bc"])
        ffnw_sb = [const.tile([128, 256], BF16, tag=f"ffnw{h}", name=f"ffnw{h}")
                   for h in range(4)]
        for h in range(4):
            nc.sync.dma_start(ffnw_sb[h][:], d["ffnw"][h])
        ffnb_sb = const.tile([128, 256], F32, tag="ffnb", name="ffnbsb")
        nc.sync.dma_start(ffnb_sb[:], d["ffnb"])

        # persistent activations (bf16)
        y_sb = {}   # dw conv outputs, filled in phase order
        q_sb = [const.tile([128, HWN], BF16, tag=f"qsb{i}", name=f"qsb{i}") for i in range(4)]
        k_sb = [const.tile([128, HWN], BF16, tag=f"ksb{i}", name=f"ksb{i}") for i in range(4)]
        vt_sb = [const.tile([128, 8 * 66], BF16, tag=f"vt{i}", name=f"vt{i}") for i in range(8)]
        ot_sb = [const.tile([128, HWN], BF16, tag=f"ot{i}", name=f"ot{i}") for i in range(4)]
        for i in range(8):
            vtv = vt_sb[i][:].rearrange("p (h c) -> p h c", c=66)
            nc.sync.dma_start(vtv[:, :, 64:65], d["vt_ones"])

        # dw diag construction on DVE: dwd[p, blk] = [128, 9*128] bf16
        dwd_sb = {}
        for ci, p in enumerate(("q", "k", "v")):
            for blk in range(2):
                t = const.tile([128, 9 * 128], BF16, tag=f"dw{p}{blk}", name=f"dwd{p}{blk}")
                i0 = ci * 18 + blk * 9
                e3 = eye_sb[:].rearrange("p (a c) -> p a c", a=1)
                w3 = dwt_sb[:, i0:i0 + 9].rearrange("p (a c) -> p a c", c=1)
                e3b, w3b = bass.broadcast_tensor_aps(e3, w3)
                nc.vector.tensor_tensor(
                    t[:].rearrange("p (a c) -> p a c", c=128), e3b, w3b,
                    op=mybir.AluOpType.mult)
                dwd_sb[p, blk] = t

        def emit_dw(p, xin, blk, pool):
            """depthwise conv for proj p, k-block blk -> y_sb[p, blk] bf16"""
            ps = pool.tile([128, HWN], F32, tag="ps", name=f"psdw{p}{blk}")
            xv = x_sb[xin, blk][:].rearrange("p (r c) -> p r c", c=34)
            for tap in range(9):
                di, dj = tap // 3, tap % 3
                lhsT = dwd_sb[p, blk][:, tap * 128:(tap + 1) * 128]
                for hf in range(2):
                    rhs = xv[:, di + hf * 16: di + hf * 16 + 16, dj: dj + 32]
                    mm(ps[:, hf * 512:(hf + 1) * 512], lhsT, rhs,
                       start=(tap == 0), stop=(tap == 8))
            y = const.tile([128, HWN], BF16, tag=f"y{p}{blk}", name=f"y{p}{blk}")
            nc.vector.tensor_copy(y[:], ps[:])
            y_sb[p, blk] = y

        def emit_pw_qk(pair, pool):
            """pointwise conv for q and k, oc block = pair (2 heads)."""
            for p, dest in (("q", q_sb), ("k", k_sb)):
                ps = pool.tile([128, HWN], F32, tag="ps", name=f"pspw{p}{pair}")
                for kc in range(2):
                    for hf in range(2):
                        mm(ps[:, hf * 512:(hf + 1) * 512],
                           w_sb[p][kc][:, pair * 128:(pair + 1) * 128],
                           y_sb[p, kc][:, hf * 512:(hf + 1) * 512],
                           start=(kc == 0), stop=(kc == 1))
                ci = 0 if p == "q" else 1
                nc.vector.tensor_scalar_add(
                    dest[pair][:], ps[:], qkb_sb[:, ci * 4 + pair: ci * 4 + pair + 1])

        def emit_pw_v(half, pool):
            """pointwise conv for v, hw blocks [2*half, 2*half+1], transposed."""
            ps = pool.tile([128, HWN], F32, tag="ps", name=f"psv{half}")
            for sub in range(2):
                mb = 2 * half + sub
                for kc in range(2):
                    mm(ps[:, sub * 512:(sub + 1) * 512],
                       y_sb["v", kc][:, mb * 128:(mb + 1) * 128],
                       w_sb["v"][kc][:], start=(kc == 0), stop=(kc == 1))
            for sub in range(2):
                mb = 2 * half + sub
                vtv = vt_sb[mb][:].rearrange("p (h c) -> p h c", c=66)
                nc.vector.tensor_copy(vtv[:, :, 0:64],
                                      ps[:, sub * 512:(sub + 1) * 512])

        # ---------------- fused conv + attention pipeline ----------------
        with tc.tile_pool(name="epool", bufs=20) as epool, \
             tc.tile_pool(name="ps", bufs=2, space="PSUM") as pspool, \
             tc.tile_pool(name="norm", bufs=2) as npool:
            e_tiles = {}   # (h, jb) -> bf16 [128, 1024]

            def emit_scores(h):
                pair, pb = h // 2, (h % 2) * 64
                for jb in range(8):
                    sp = pspool.tile([128, HWN], F32, tag="ps", name="sp")
                    for hf in range(2):
                        mm(sp[:, hf * 512:(hf + 1) * 512],
                           k_sb[pair][pb:pb + 64, jb * 128:(jb + 1) * 128],
                           q_sb[pair][pb:pb + 64, hf * 512:(hf + 1) * 512],
                           start=True, stop=True)
                    e = epool.tile([128, HWN], BF16, tag="e", name="e")
                    nc.scalar.activation(e[:], sp[:],
                                         mybir.ActivationFunctionType.Exp,
                                         scale=SCALE)
                    e_tiles[h, jb] = e

            def emit_av(h, opool, tag):
                oacc = opool.tile([65, HWN], F32, tag=tag, name=f"oacc{h}")
                for jb in range(8):
                    e = e_tiles[h, jb]
                    for hf in range(2):
                        mm(oacc[:, hf * 512:(hf + 1) * 512],
                           vt_sb[jb][:, 66 * h: 66 * h + 65],
                           e[:, hf * 512:(hf + 1) * 512],
                           start=(jb == 0), stop=(jb == 7))
                return oacc

            def emit_norm(h, oacc, opool, tag):
                pair, pb = h // 2, (h % 2) * 64
                # reciprocal of the denominator row, straight from PSUM
                rrow = npool.tile([1, HWN], BF16, tag="rrow", name="rrow")
                nc.vector.reciprocal(rrow[:], oacc[64:65, :])
                o_un = npool.tile([64, HWN], F32, tag="oun", name="o_un")
                nc.vector.tensor_copy(o_un[:], oacc[0:64, :])
                # rank-1 broadcast of rrow across 64 partitions (reuses oacc bank)
                bc = opool.tile([64, HWN], F32, tag=tag, name=f"bc{h}")
                for hf in range(2):
                    mm(bc[:, hf * 512:(hf + 1) * 512],
                       ones_bc_sb[0:1, :],
                       rrow[0:1, hf * 512:(hf + 1) * 512],
                       start=True, stop=True)
                if pb == 0:
                    nc.vector.tensor_mul(ot_sb[pair][0:64, :], o_un[:], bc[:])
                else:
                    # DVE lanes can't cross partitions: normalize at base 0,
                    # then DMA the finished bf16 slice up to partitions 64:128
                    tmp = npool.tile([64, HWN], BF16, tag="otmp", name="ot_tmp")
                    nc.vector.tensor_mul(tmp[:], o_un[:], bc[:])
                    nc.sync.dma_start(ot_sb[pair][64:128, :], tmp[:])

            with tc.tile_pool(name="oaccB", bufs=1, space="PSUM") as opB:
                with tc.tile_pool(name="oaccA", bufs=1, space="PSUM") as opA:
                    op = {0: (opA, "oA"), 1: (opB, "oB")}

                    # conv front: q,k dw then first two head-pairs of pw+scores
                    for blk in range(2):
                        emit_dw("q", "q", blk, pspool)
                    for blk in range(2):
                        emit_dw("k", "kv", blk, pspool)
                    emit_pw_qk(0, pspool)
                    emit_scores(0)
                    emit_pw_qk(1, pspool)
                    emit_scores(1)
                    # v path
                    for blk in range(2):
                        emit_dw("v", "kv", blk, pspool)
                    for half in range(4):
                        emit_pw_v(half, pspool)

                    oaccs = {}
                    oaccs[0] = emit_av(0, *op[0])
                    emit_scores(2)
                    oaccs[1] = emit_av(1, *op[1])
                    emit_pw_qk(2, pspool)
                    emit_scores(3)
                    emit_norm(0, oaccs[0], *op[0])
                    oaccs[2] = emit_av(2, *op[0])
                    emit_pw_qk(3, pspool)
                    emit_scores(4)
                    emit_norm(1, oaccs[1], *op[1])
                    oaccs[3] = emit_av(3, *op[1])
                    emit_scores(5)
                    emit_norm(2, oaccs[2], *op[0])
                    oaccs[4] = emit_av(4, *op[0])
                    emit_scores(7)
                    emit_norm(3, oaccs[3], *op[1])
                    oaccs[5] = emit_av(5, *op[1])
                    emit_scores(6)
                    emit_norm(4, oaccs[4], *op[0])
                    oaccs[7] = emit_av(7, *op[0])
                    emit_norm(5, oaccs[5], *op[1])
                    emit_norm(7, oaccs[7], *op[0])
                # oaccA closed: 2 PSUM banks free for early ffn blocks
                with tc.tile_pool(name="psf", bufs=1, space="PSUM") as psf:
                    # one open accumulation group per 2KB PSUM bank: one nb
                    # block per tile (use half the bank, 256 of 512 cols)
                    f_t = [psf.tile([128, 512], F32, tag=f"f{i}", name=f"psf{i}")
                           for i in range(2)]
                    for nb in range(2):      # nb 0..1: kc 0..2 pre-issued
                        for kc in range(3):
                            mm(f_t[nb][:, 0:256],
                               ot_sb[kc][:, nb * 128:(nb + 1) * 128],
                               ffnw_sb[kc][:], start=(kc == 0), stop=False)
                    oaccs[6] = emit_av(6, *op[1])
                    emit_norm(6, oaccs[6], *op[1])
                    with tc.tile_pool(name="fout", bufs=4) as fpool:
                        def fin(nb):
                            fo = fpool.tile([128, 256], F32, tag="fo", name="fo")
                            nc.vector.tensor_tensor(
                                fo[:], f_t[nb % 2][:, 0:256],
                                ffnb_sb[:], op=mybir.AluOpType.add)
                            nc.sync.dma_start(
                                d["out"][nb * 128:(nb + 1) * 128, :], fo[:])
                        for nb in range(2):
                            mm(f_t[nb][:, 0:256],
                               ot_sb[3][:, nb * 128:(nb + 1) * 128],
                               ffnw_sb[3][:], start=False, stop=True)
                            fin(nb)
                        for nb in range(2, 8):   # remaining blocks, 2-bank rotation
                            for kc in range(4):
                                mm(f_t[nb % 2][:, 0:256],
                                   ot_sb[kc][:, nb * 128:(nb + 1) * 128],
                                   ffnw_sb[kc][:], start=(kc == 0), stop=(kc == 3))
                            fin(nb)


def _build():
    nc = bacc.Bacc("TRN2", target_bir_lowering=False, debug=False)
    with tile.TileContext(nc) as tc:
        _emit(nc, tc)
    nc.compile()
    return nc


# ----------------------------------------------------------------- host code

def _host_shared(inputs):
    g = lambda n: np.asarray(inputs[n], dtype=np.float32)
    d = {}
    dw_effs = []
    qk_bias_cols = []
    for ci, p in enumerate(("q", "k", "v")):
        a = g(f"{p}_bn_g") / np.sqrt(g(f"{p}_bn_v") + EPS)          # [256]
        dw_eff = g(f"{p}_dw_w")[:, 0] * a[:, None, None]            # [256,3,3]
        beta = a * g(f"{p}_dw_b") + g(f"{p}_bn_b") - a * g(f"{p}_bn_m")
        pw = g(f"{p}_pw_w")[:, :, 0, 0]                             # [512,256]
        bias = g(f"{p}_pw_b") + pw @ beta                           # [512]
        dw_effs.append(dw_eff)
        d[f"w{p}"] = np.ascontiguousarray(pw.T.reshape(2, 128, 512)).astype(np.float32)
        if p == "v":
            vbias = bias                                            # [512]
        else:
            qk_bias_cols.append(bias)
    qkb = np.zeros((128, 8), np.float32)
    for ci in range(2):
        for mb in range(4):
            qkb[:, ci * 4 + mb] = qk_bias_cols[ci][mb * 128:(mb + 1) * 128]
    d["qk_bias"] = qkb
    d["eye"] = np.eye(128, dtype=np.float32)
    dwtm = np.zeros((128, 54), np.float32)
    for ci in range(3):
        for blk in range(2):
            for t in range(9):
                dwtm[:, ci * 18 + blk * 9 + t] = dw_effs[ci][blk * 128:(blk + 1) * 128, t // 3, t % 3]
    d["dwt"] = dwtm
    d["vt_ones"] = np.ones((128, 8, 1), np.float32)
    d["ones_bc"] = np.ones((1, 64), np.float32)
    d["ffnw"] = np.ascontiguousarray(g("ffn_w").T.reshape(4, 128, 256)).astype(np.float32)
    # v pointwise bias folded through the ffn (exact): out += (ffn_w @ v_bias)
    ffnb_eff = g("ffn_b") + g("ffn_w") @ vbias                      # [256]
    d["ffnb"] = np.broadcast_to(ffnb_eff, (128, 256)).copy()
    # bf16 conversion for the matmul operands
    import ml_dtypes
    for k in ("wq", "wk", "wv", "eye", "dwt", "vt_ones", "ones_bc", "ffnw"):
        d[k] = d[k].astype(ml_dtypes.bfloat16)
    return d


def _host_x(feat):
    # [1024, 256] -> padded transposed [2, 128, 34*34] bf16
    import ml_dtypes
    xt = np.ascontiguousarray(np.asarray(feat, np.float32).T).reshape(2, 128, 32, 32)
    xp = np.zeros((2, 128, 34, 34), np.float32)
    xp[:, :, 1:33, 1:33] = xt
    return xp.reshape(2, 128, PAD).astype(ml_dtypes.bfloat16)


def make_in_maps(inputs):
    shared = _host_shared(inputs)
    f1 = np.asarray(inputs["features1"], dtype=np.float32)
    f2 = np.asarray(inputs["features2"], dtype=np.float32)
    maps = []
    for b in range(B):
        m = dict(shared)
        m["xq"] = _host_x(f1[b])
        m["xkv"] = _host_x(f2[b])
        maps.append(m)
    return maps


def get_nc():
    if "nc" not in _CACHE:
        _CACHE["nc"] = _build()
    return _CACHE["nc"]


def kernel(**inputs):
    nc = get_nc()
    in_maps = make_in_maps(inputs)
    res = run_bass_kernel_spmd(nc, in_maps, list(range(B)))
    return np.stack([res.results[i]["out"] for i in range(B)]).astype(np.float32)


# revision 19
# speedup vs baseline: 1.5346x; 1.4179x over previous
"""Trainium2 Bass kernel for nn_ConvolutionAttention (v2: bf16 pipeline).

Reference computation (per batch element b of B=8):
  x1 = features1[b] as [C=256, 32, 32];  x2 = features2[b] likewise
  q = pw(bn(dw3x3(x1)));  k = pw(bn(dw3x3(x2)));  v same as k w/ own weights
  per head h (8 heads, dh=64): attn = softmax(q_h k_h^T / 8);  o_h = attn v_h
  out[b] = concat_h(o_h) @ ffn_w.T + ffn_b      -> [1024, 256]

Sharding: pure data-parallel over batch; core i computes batch element i.

v2 design notes (vs the f32r baseline):
  - all matmul operands bf16: f32r streams 2 cycles/col on HW (fp32_mode=HIGH),
    bf16 streams 1 col/cycle -> ~2x on every matmul.
  - per-head attention (not head-pairs), software-pipelined against the
    convolutions so ACT-engine EXP (the 73us serial floor) starts ~20us in.
  - k pointwise bias kept (free via tensor_scalar on DVE); v pointwise bias
    folded through ffn_w into ffn_b on host (exact); ffn bias applied as a
    DVE tensor_tensor add with a host-replicated [128,256] tile (kills the
    K=1 bias matmuls).
  - softmax normalize: reciprocal straight off the PSUM denominator row
    ([1,1024] DVE op), rank-1 ones-matmul broadcast, DVE multiply. No SBUF
    DMA round trips.
  - ffn for nb-blocks 0..3 kc 0..2 pre-issued before the last head's av
    completes (PSUM banks freed by closing the A accumulator pool early).
"""

import numpy as np

import concourse.bass as bass
import concourse.bacc as bacc
import concourse.tile as tile
from concourse import mybir
from concourse.bass_utils import run_bass_kernel_spmd

F32 = mybir.dt.float32
BF16 = mybir.dt.bfloat16

B, C, HWN, H, W = 8, 256, 1024, 32, 32
HEADS, DH, OC = 8, 64, 512
SCALE = DH ** -0.5
EPS = 1e-5
PAD = 34 * 34  # 1156

_CACHE = {}


# ----------------------------------------------------------------- device code

def _emit(nc, tc):
    # ---- DRAM I/O ----
    xq = nc.dram_tensor("xq", [2, 128, PAD], BF16, kind="ExternalInput").ap()
    xkv = nc.dram_tensor("xkv", [2, 128, PAD], BF16, kind="ExternalInput").ap()
    eye = nc.dram_tensor("eye", [128, 128], BF16, kind="ExternalInput").ap()
    dwt = nc.dram_tensor("dwt", [128, 54], BF16, kind="ExternalInput").ap()
    wq = nc.dram_tensor("wq", [2, 128, 512], BF16, kind="ExternalInput").ap()
    wk = nc.dram_tensor("wk", [2, 128, 512], BF16, kind="ExternalInput").ap()
    wv = nc.dram_tensor("wv", [2, 128, 512], BF16, kind="ExternalInput").ap()
    qk_bias = nc.dram_tensor("qk_bias", [128, 8], F32, kind="ExternalInput").ap()
    vt_ones = nc.dram_tensor("vt_ones", [128, 8, 1], BF16, kind="ExternalInput").ap()
    ones_bc = nc.dram_tensor("ones_bc", [1, 64], BF16, kind="ExternalInput").ap()
    ffnw = nc.dram_tensor("ffnw", [4, 128, 256], BF16, kind="ExternalInput").ap()
    ffnb = nc.dram_tensor("ffnb", [128, 256], F32, kind="ExternalInput").ap()
    out = nc.dram_tensor("out", [HWN, C], F32, kind="ExternalOutput").ap()

    with nc.allow_low_precision(reason="bf16 matmul pipeline"):
        _emit_body(nc, tc, dict(xq=xq, xkv=xkv, eye=eye, dwt=dwt, wq=wq, wk=wk,
                                wv=wv, qk_bias=qk_bias, vt_ones=vt_ones,
                                ones_bc=ones_bc, ffnw=ffnw, ffnb=ffnb, out=out))


def _emit_body(nc, tc, d):
    mm = nc.tensor.matmul
    wmap = {"q": d["wq"], "k": d["wk"], "v": d["wv"]}

    with tc.tile_pool(name="const", bufs=1) as const:
        # small consts first (cheap DMAs, needed early)
        eye_sb = const.tile([128, 128], BF16, tag="eye", name="eye_sb")
        nc.sync.dma_start(eye_sb[:], d["eye"])
        dwt_sb = const.tile([128, 54], BF16, tag="dwt", name="dwt_sb")
        nc.sync.dma_start(dwt_sb[:], d["dwt"])
        # activations: xq first (critical path), then xkv
        x_sb = {}
        for nm, src in (("q", d["xq"]), ("kv", d["xkv"])):
            for blk in range(2):
                t = const.tile([128, PAD], BF16, tag=f"x{nm}{blk}", name=f"x{nm}{blk}")
                nc.sync.dma_start(t[:], src[blk])
                x_sb[nm, blk] = t
        # weights needed from ~10us on
        w_sb = {p: [const.tile([128, 512], BF16, tag=f"w{p}{kc}", name=f"w{p}{kc}")
                    for kc in range(2)] for p in ("q", "k", "v")}
        for p in ("q", "k", "v"):
            for kc in range(2):
                nc.sync.dma_start(w_sb[p][kc][:], wmap[p][kc])
        qkb_sb = const.tile([128, 8], F32, tag="qkb", name="qkb")
        nc.sync.dma_start(qkb_sb[:], d["qk_bias"])
        ones_bc_sb = const.tile([1, 64], BF16, tag="onesbc", name="ones_bc_sb")
        nc.sync.dma_start(ones_bc_sb[:], d["ones_bc"])
        ffnw_sb = [const.tile([128, 256], BF16, tag=f"ffnw{h}", name=f"ffnw{h}")
                   for h in range(4)]
        for h in range(4):
            nc.sync.dma_start(ffnw_sb[h][:], d["ffnw"][h])
        ffnb_sb = const.tile([128, 256], F32, tag="ffnb", name="ffnbsb")
        nc.sync.dma_start(ffnb_sb[:], d["ffnb"])

        # persistent activations (bf16)
        y_sb = {}   # dw conv outputs, filled in phase order
        q_sb = [const.tile([128, HWN], BF16, tag=f"qsb{i}", name=f"qsb{i}") for i in range(4)]
        k_sb = [const.tile([128, HWN], BF16, tag=f"ksb{i}", name=f"ksb{i}") for i in range(4)]
        vt_sb = [const.tile([128, 8 * 66], BF16, tag=f"vt{i}", name=f"vt{i}") for i in range(8)]
        ot_sb = [const.tile([128, HWN], BF16, tag=f"ot{i}", name=f"ot{i}") for i in range(4)]
        for i in range(8):
            vtv = vt_sb[i][:].rearrange("p (h c) -> p h c", c=66)
            nc.sync.dma_start(vtv[:, :, 64:65], d["vt_ones"])

        # dw diag construction on DVE: dwd[p, blk] = [128, 9*128] bf16
        dwd_sb = {}
        for ci, p in enumerate(("q", "k", "v")):
            for blk in range(2):
                t = const.tile([128, 9 * 128], BF16, tag=f"dw{p}{blk}", name=f"dwd{p}{blk}")
                i0 = ci * 18 + blk * 9
                e3 = eye_sb[:].rearrange("p (a c) -> p a c", a=1)
                w3 = dwt_sb[:, i0:i0 + 9].rearrange("p (a c) -> p a c", c=1)
                e3b, w3b = bass.broadcast_tensor_aps(e3, w3)
                nc.vector.tensor_tensor(
                    t[:].rearrange("p (a c) -> p a c", c=128), e3b, w3b,
                    op=mybir.AluOpType.mult)
                dwd_sb[p, blk] = t

        def emit_dw(p, xin, blk, pool):
            """depthwise conv for proj p, k-block blk -> y_sb[p, blk] bf16"""
            ps = pool.tile([128, HWN], F32, tag="ps", name=f"psdw{p}{blk}")
            xv = x_sb[xin, blk][:].rearrange("p (r c) -> p r c", c=34)
            for tap in range(9):
                di, dj = tap // 3, tap % 3
                lhsT = dwd_sb[p, blk][:, tap * 128:(tap + 1) * 128]
                mm(ps[:], lhsT, xv[:, di: di + 32, dj: dj + 32],
                   start=(tap == 0), stop=(tap == 8))
            y = const.tile([128, HWN], BF16, tag=f"y{p}{blk}", name=f"y{p}{blk}")
            nc.vector.tensor_copy(y[:], ps[:])
            y_sb[p, blk] = y

        def emit_pw_qk(pair, pool):
            """pointwise conv for q and k, oc block = pair (2 heads)."""
            for p, dest in (("q", q_sb), ("k", k_sb)):
                ps = pool.tile([128, HWN], F32, tag="ps", name=f"pspw{p}{pair}")
                for kc in range(2):
                    mm(ps[:], w_sb[p][kc][:, pair * 128:(pair + 1) * 128],
                       y_sb[p, kc][:], start=(kc == 0), stop=(kc == 1))
                ci = 0 if p == "q" else 1
                nc.vector.tensor_scalar_add(
                    dest[pair][:], ps[:], qkb_sb[:, ci * 4 + pair: ci * 4 + pair + 1])

        def emit_pw_v(half, pool):
            """pointwise conv for v, hw blocks [2*half, 2*half+1], transposed."""
            ps = pool.tile([128, HWN], F32, tag="ps", name=f"psv{half}")
            for sub in range(2):
                mb = 2 * half + sub
                for kc in range(2):
                    mm(ps[:, sub * 512:(sub + 1) * 512],
                       y_sb["v", kc][:, mb * 128:(mb + 1) * 128],
                       w_sb["v"][kc][:], start=(kc == 0), stop=(kc == 1))
            for sub in range(2):
                mb = 2 * half + sub
                vtv = vt_sb[mb][:].rearrange("p (h c) -> p h c", c=66)
                nc.vector.tensor_copy(vtv[:, :, 0:64],
                                      ps[:, sub * 512:(sub + 1) * 512])

        # ---------------- fused conv + attention pipeline ----------------
        # ACT-engine EXP (64 x 1.15us) is the serial floor; scores for head
        # order[i+2] are jb-interleaved with av for head order[i] so the PE
        # tracks ACT pacing, and the v-path convs run as fillers inside the
        # EXP-paced h0/h1 scores stretch (their PSUM pool closes before the
        # attention accumulators open).
        from contextlib import ExitStack
        order = [0, 1, 2, 3, 4, 5, 7, 6]
        with tc.tile_pool(name="epool", bufs=20) as epool, \
             tc.tile_pool(name="ps", bufs=2, space="PSUM") as pspool, \
             tc.tile_pool(name="norm", bufs=2) as npool:
            e_tiles = {}

            def emit_scores_jb(h, jb):
                pair, pb = h // 2, (h % 2) * 64
                sp = pspool.tile([128, HWN], F32, tag="ps", name="sp")
                for hf in range(2):
                    mm(sp[:, hf * 512:(hf + 1) * 512],
                       k_sb[pair][pb:pb + 64, jb * 128:(jb + 1) * 128],
                       q_sb[pair][pb:pb + 64, hf * 512:(hf + 1) * 512],
                       start=True, stop=True)
                e = epool.tile([128, HWN], BF16, tag="e", name="e")
                nc.scalar.activation(e[:], sp[:],
                                     mybir.ActivationFunctionType.Exp,
                                     scale=SCALE)
                e_tiles[h, jb] = e

            def emit_av_jb(h, jb, oacc):
                for hf in range(2):
                    mm(oacc[:, hf * 512:(hf + 1) * 512],
                       vt_sb[jb][:, 66 * h: 66 * h + 65],
                       e_tiles[h, jb][:, hf * 512:(hf + 1) * 512],
                       start=(jb == 0), stop=(jb == 7))

            def emit_norm(h, oacc, opool, tag):
                pair, pb = h // 2, (h % 2) * 64
                o_un = npool.tile([65, HWN], F32, tag="oun", name="o_un")
                nc.vector.tensor_copy(o_un[:], oacc[:])
                # reciprocal of the denominator row: reshape to [64,16] via DMA
                # (a [1,1024] single-lane DVE reciprocal measures 6.5us)
                csp = npool.tile([64, 16], F32, tag="csp", name="csp")
                nc.sync.dma_start(
                    csp[:], o_un[64:65, :].rearrange("p (a b) -> p a b", b=16))
                csr = npool.tile([64, 16], BF16, tag="csr", name="csr")
                nc.vector.reciprocal(csr[:], csp[:])
                rrow = npool.tile([1, HWN], BF16, tag="rrow", name="rrow")
                nc.sync.dma_start(
                    rrow[:].rearrange("p (a b) -> p a b", b=16), csr[:])
                # rank-1 broadcast of rrow across 64 partitions (reuses oacc bank)
                bc = opool.tile([64, HWN], F32, tag=tag, name=f"bc{h}")
                for hf in range(2):
                    mm(bc[:, hf * 512:(hf + 1) * 512], ones_bc_sb[0:1, :],
                       rrow[0:1, hf * 512:(hf + 1) * 512], start=True, stop=True)
                if pb == 0:
                    nc.vector.tensor_mul(ot_sb[pair][0:64, :], o_un[0:64, :], bc[:])
                else:
                    # DVE lanes can't cross partitions: normalize at base 0,
                    # then DMA the finished bf16 slice up to partitions 64:128
                    tmp = npool.tile([64, HWN], BF16, tag="otmp", name="ot_tmp")
                    nc.vector.tensor_mul(tmp[:], o_un[0:64, :], bc[:])
                    nc.sync.dma_start(ot_sb[pair][64:128, :], tmp[:])

            def gen_dw(p, xin, blk, pool):
                ps = pool.tile([128, HWN], F32, tag="ps", name=f"psdw{p}{blk}")
                xv = x_sb[xin, blk][:].rearrange("p (r c) -> p r c", c=34)
                for tap in range(9):
                    di, dj = tap // 3, tap % 3
                    lhsT = dwd_sb[p, blk][:, tap * 128:(tap + 1) * 128]
                    for hf in range(2):
                        rhs = xv[:, di + hf * 16: di + hf * 16 + 16, dj: dj + 32]
                        mm(ps[:, hf * 512:(hf + 1) * 512], lhsT, rhs,
                           start=(tap == 0), stop=(tap == 8))
                    yield
                y = const.tile([128, HWN], BF16, tag=f"y{p}{blk}", name=f"y{p}{blk}")
                nc.vector.tensor_copy(y[:], ps[:])
                y_sb[p, blk] = y

            def gen_pw_qk(pair, pool):
                for p, dest in (("q", q_sb), ("k", k_sb)):
                    ps = pool.tile([128, HWN], F32, tag="ps", name=f"pspw{p}{pair}")
                    for kc in range(2):
                        for hf in range(2):
                            mm(ps[:, hf * 512:(hf + 1) * 512],
                               w_sb[p][kc][:, pair * 128:(pair + 1) * 128],
                               y_sb[p, kc][:, hf * 512:(hf + 1) * 512],
                               start=(kc == 0), stop=(kc == 1))
                        yield
                    ci = 0 if p == "q" else 1
                    nc.vector.tensor_scalar_add(
                        dest[pair][:], ps[:],
                        qkb_sb[:, ci * 4 + pair: ci * 4 + pair + 1])

            def gen_pw_v(half, pool):
                ps = pool.tile([128, HWN], F32, tag="ps", name=f"psv{half}")
                for sub in range(2):
                    mb = 2 * half + sub
                    for kc in range(2):
                        mm(ps[:, sub * 512:(sub + 1) * 512],
                           y_sb["v", kc][:, mb * 128:(mb + 1) * 128],
                           w_sb["v"][kc][:], start=(kc == 0), stop=(kc == 1))
                    yield
                for sub in range(2):
                    mb = 2 * half + sub
                    vtv = vt_sb[mb][:].rearrange("p (h c) -> p h c", c=66)
                    nc.vector.tensor_copy(vtv[:, :, 0:64],
                                          ps[:, sub * 512:(sub + 1) * 512])

            # --- conv stage ---
            with tc.tile_pool(name="psdw", bufs=2, space="PSUM") as psdw:
                for blk in range(2):
                    for _ in gen_dw("q", "q", blk, psdw):
                        pass
                for blk in range(2):
                    for _ in gen_dw("k", "kv", blk, psdw):
                        pass
                for _ in gen_pw_qk(0, psdw):
                    pass
                # v path + remaining qk pointwise, stepped as fillers between
                # the EXP-paced h0/h1 scores
                pending = [gen_dw("v", "kv", 0, psdw), gen_pw_qk(1, psdw),
                           gen_dw("v", "kv", 1, psdw), gen_pw_qk(2, psdw),
                           gen_pw_qk(3, psdw)] + \
                          [gen_pw_v(half, psdw) for half in range(4)]

                def step_fill(n):
                    while n > 0 and pending:
                        try:
                            next(pending[0])
                            n -= 1
                        except StopIteration:
                            pending.pop(0)

                for h in order[:2]:
                    for jb in range(8):
                        emit_scores_jb(h, jb)
                        step_fill(2)
                step_fill(10 ** 6)

            # --- attention stage ---
            stA = ExitStack()
            with tc.tile_pool(name="oaccB", bufs=1, space="PSUM") as opB:
                opA = stA.enter_context(
                    tc.tile_pool(name="oaccA", bufs=1, space="PSUM"))
                op = {0: (opA, "oA"), 1: (opB, "oB")}
                for idx in range(7):           # heads 0,1,2,3,4,5,7
                    h = order[idx]
                    opool, tag = op[idx % 2]
                    oacc = opool.tile([65, HWN], F32, tag=tag, name=f"oacc{h}")
                    h2 = order[idx + 2] if idx + 2 < 8 else None
                    for jb in range(8):
                        if h2 is not None:
                            emit_scores_jb(h2, jb)
                        emit_av_jb(h, jb, oacc)
                    emit_norm(h, oacc, opool, tag)
                stA.close()   # frees 2 banks for the early ffn blocks
                with tc.tile_pool(name="psf", bufs=1, space="PSUM") as psf:
                    # one open accumulation group per 2KB PSUM bank
                    f_t = [psf.tile([128, 512], F32, tag=f"f{i}", name=f"psf{i}")
                           for i in range(2)]
                    for nb in range(2):        # nb 0..1: kc 0..2 pre-issued
                        for kc in range(3):
                            mm(f_t[nb][:, 0:256],
                               ot_sb[kc][:, nb * 128:(nb + 1) * 128],
                               ffnw_sb[kc][:], start=(kc == 0), stop=False)
                    # last head (6, even: short normalize chain)
                    oacc6 = opB.tile([65, HWN], F32, tag="oB", name="oacc6")
                    for jb in range(8):
                        emit_av_jb(6, jb, oacc6)
                    emit_norm(6, oacc6, opB, "oB")
                    with tc.tile_pool(name="fout", bufs=4) as fpool:
                        def fin(nb):
                            fo = fpool.tile([128, 256], F32, tag="fo", name="fo")
                            nc.vector.tensor_tensor(
                                fo[:], f_t[nb % 2][:, 0:256],
                                ffnb_sb[:], op=mybir.AluOpType.add)
                            nc.sync.dma_start(
                                d["out"][nb * 128:(nb + 1) * 128, :], fo[:])
                        for nb in range(2):
                            mm(f_t[nb][:, 0:256],
                               ot_sb[3][:, nb * 128:(nb + 1) * 128],
                               ffnw_sb[3][:], start=False, stop=True)
                            fin(nb)
                        for nb in range(2, 8):
                            for kc in range(4):
                                mm(f_t[nb % 2][:, 0:256],
                                   ot_sb[kc][:, nb * 128:(nb + 1) * 128],
                                   ffnw_sb[kc][:], start=(kc == 0), stop=(kc == 3))
                            fin(nb)


def _build():
    nc = bacc.Bacc("TRN2", target_bir_lowering=False, debug=False)
    with tile.TileContext(nc) as tc:
        _emit(nc, tc)
    nc.compile()
    return nc


# ----------------------------------------------------------------- host code

def _host_shared(inputs):
    g = lambda n: np.asarray(inputs[n], dtype=np.float32)
    d = {}
    dw_effs = []
    qk_bias_cols = []
    for ci, p in enumerate(("q", "k", "v")):
        a = g(f"{p}_bn_g") / np.sqrt(g(f"{p}_bn_v") + EPS)          # [256]
        dw_eff = g(f"{p}_dw_w")[:, 0] * a[:, None, None]            # [256,3,3]
        beta = a * g(f"{p}_dw_b") + g(f"{p}_bn_b") - a * g(f"{p}_bn_m")
        pw = g(f"{p}_pw_w")[:, :, 0, 0]                             # [512,256]
        bias = g(f"{p}_pw_b") + pw @ beta                           # [512]
        dw_effs.append(dw_eff)
        d[f"w{p}"] = np.ascontiguousarray(pw.T.reshape(2, 128, 512)).astype(np.float32)
        if p == "v":
            vbias = bias                                            # [512]
        else:
            qk_bias_cols.append(bias)
    qkb = np.zeros((128, 8), np.float32)
    for ci in range(2):
        for mb in range(4):
            qkb[:, ci * 4 + mb] = qk_bias_cols[ci][mb * 128:(mb + 1) * 128]
    d["qk_bias"] = qkb
    d["eye"] = np.eye(128, dtype=np.float32)
    dwtm = np.zeros((128, 54), np.float32)
    for ci in range(3):
        for blk in range(2):
            for t in range(9):
                dwtm[:, ci * 18 + blk * 9 + t] = dw_effs[ci][blk * 128:(blk + 1) * 128, t // 3, t % 3]
    d["dwt"] = dwtm
    d["vt_ones"] = np.ones((128, 8, 1), np.float32)
    d["ones_bc"] = np.ones((1, 64), np.float32)
    d["ffnw"] = np.ascontiguousarray(g("ffn_w").T.reshape(4, 128, 256)).astype(np.float32)
    # v pointwise bias folded through the ffn (exact): out += (ffn_w @ v_bias)
    ffnb_eff = g("ffn_b") + g("ffn_w") @ vbias                      # [256]
    d["ffnb"] = np.broadcast_to(ffnb_eff, (128, 256)).copy()
    # bf16 conversion for the matmul operands
    import ml_dtypes
    for k in ("wq", "wk", "wv", "eye", "dwt", "vt_ones", "ones_bc", "ffnw"):
        d[k] = d[k].astype(ml_dtypes.bfloat16)
    return d


def _host_x(feat):
    # [1024, 256] -> padded transposed [2, 128, 34*34] bf16
    import ml_dtypes
    xt = np.ascontiguousarray(np.asarray(feat, np.float32).T).reshape(2, 128, 32, 32)
    xp = np.zeros((2, 128, 34, 34), np.float32)
    xp[:, :, 1:33, 1:33] = xt
    return xp.reshape(2, 128, PAD).astype(ml_dtypes.bfloat16)


def make_in_maps(inputs):
    shared = _host_shared(inputs)
    f1 = np.asarray(inputs["features1"], dtype=np.float32)
    f2 = np.asarray(inputs["features2"], dtype=np.float32)
    maps = []
    for b in range(B):
        m = dict(shared)
        m["xq"] = _host_x(f1[b])
        m["xkv"] = _host_x(f2[b])
        maps.append(m)
    return maps


def get_nc():
    if "nc" not in _CACHE:
        _CACHE["nc"] = _build()
    return _CACHE["nc"]


def kernel(**inputs):
    nc = get_nc()
    in_maps = make_in_maps(inputs)
    res = run_bass_kernel_spmd(nc, in_maps, list(range(B)))
    return np.stack([res.results[i]["out"] for i in range(B)]).astype(np.float32)


# revision 21
# speedup vs baseline: 1.6881x; 1.1000x over previous
"""Trainium2 Bass kernel for nn_ConvolutionAttention (v2: bf16 pipeline).

Reference computation (per batch element b of B=8):
  x1 = features1[b] as [C=256, 32, 32];  x2 = features2[b] likewise
  q = pw(bn(dw3x3(x1)));  k = pw(bn(dw3x3(x2)));  v same as k w/ own weights
  per head h (8 heads, dh=64): attn = softmax(q_h k_h^T / 8);  o_h = attn v_h
  out[b] = concat_h(o_h) @ ffn_w.T + ffn_b      -> [1024, 256]

Sharding: pure data-parallel over batch; core i computes batch element i.

v2 design notes (vs the f32r baseline):
  - all matmul operands bf16: f32r streams 2 cycles/col on HW (fp32_mode=HIGH),
    bf16 streams 1 col/cycle -> ~2x on every matmul.
  - per-head attention (not head-pairs), software-pipelined against the
    convolutions so ACT-engine EXP (the 73us serial floor) starts ~20us in.
  - k pointwise bias kept (free via tensor_scalar on DVE); v pointwise bias
    folded through ffn_w into ffn_b on host (exact); ffn bias applied as a
    DVE tensor_tensor add with a host-replicated [128,256] tile (kills the
    K=1 bias matmuls).
  - softmax normalize: reciprocal straight off the PSUM denominator row
    ([1,1024] DVE op), rank-1 ones-matmul broadcast, DVE multiply. No SBUF
    DMA round trips.
  - ffn for nb-blocks 0..3 kc 0..2 pre-issued before the last head's av
    completes (PSUM banks freed by closing the A accumulator pool early).
"""

import numpy as np

import concourse.bass as bass
import concourse.bacc as bacc
import concourse.tile as tile
from concourse import mybir
from concourse.bass_utils import run_bass_kernel_spmd

F32 = mybir.dt.float32
BF16 = mybir.dt.bfloat16
FP8 = mybir.dt.float8e4

# fp8 DoubleRow tap pairing for the 3x3 depthwise conv: pairs with constant
# window offset delta, plus the leftover 9th tap
DR_PAIRS = [(0, 1), (3, 4), (6, 7), (2, 5)]

B, C, HWN, H, W = 8, 256, 1024, 32, 32
HEADS, DH, OC = 8, 64, 512
SCALE = DH ** -0.5
EPS = 1e-5
PAD = 34 * 34  # 1156

_CACHE = {}


# ----------------------------------------------------------------- device code

def _emit(nc, tc):
    # ---- DRAM I/O ----
    xq8 = nc.dram_tensor("xq8", [2, 128, PAD], FP8, kind="ExternalInput").ap()
    xkv8 = nc.dram_tensor("xkv8", [2, 128, PAD], FP8, kind="ExternalInput").ap()
    dwq8 = nc.dram_tensor("dwq8", [2, 128, 1152], FP8, kind="ExternalInput").ap()
    dwk8 = nc.dram_tensor("dwk8", [2, 128, 1152], FP8, kind="ExternalInput").ap()
    xkv = nc.dram_tensor("xkv", [2, 128, PAD], BF16, kind="ExternalInput").ap()
    eye = nc.dram_tensor("eye", [128, 128], BF16, kind="ExternalInput").ap()
    dwt = nc.dram_tensor("dwt", [128, 54], BF16, kind="ExternalInput").ap()
    wq = nc.dram_tensor("wq", [2, 128, 512], BF16, kind="ExternalInput").ap()
    wk = nc.dram_tensor("wk", [2, 128, 512], BF16, kind="ExternalInput").ap()
    wv = nc.dram_tensor("wv", [2, 128, 512], BF16, kind="ExternalInput").ap()
    qk_bias = nc.dram_tensor("qk_bias", [128, 8], F32, kind="ExternalInput").ap()
    vt_ones = nc.dram_tensor("vt_ones", [128, 8, 1], BF16, kind="ExternalInput").ap()
    ones_bc = nc.dram_tensor("ones_bc", [1, 64], BF16, kind="ExternalInput").ap()
    ffnw = nc.dram_tensor("ffnw", [4, 128, 256], BF16, kind="ExternalInput").ap()
    ffnb = nc.dram_tensor("ffnb", [128, 256], F32, kind="ExternalInput").ap()
    out = nc.dram_tensor("out", [HWN, C], F32, kind="ExternalOutput").ap()

    with nc.allow_low_precision(reason="bf16 matmul pipeline"):
        _emit_body(nc, tc, dict(xq8=xq8, xkv8=xkv8, dwq8=dwq8, dwk8=dwk8,
                                xkv=xkv, eye=eye, dwt=dwt, wq=wq, wk=wk,
                                wv=wv, qk_bias=qk_bias, vt_ones=vt_ones,
                                ones_bc=ones_bc, ffnw=ffnw, ffnb=ffnb, out=out))


def _emit_body(nc, tc, d):
    mm = nc.tensor.matmul
    wmap = {"q": d["wq"], "k": d["wk"], "v": d["wv"]}

    with tc.tile_pool(name="const", bufs=1) as const:
        # fp8 q/k-path inputs first: they gate the first matmul
        x8_sb, dwd8_sb = {}, {}
        for blk in range(2):
            t = const.tile([128, PAD], FP8, tag=f"x8q{blk}", name=f"x8q{blk}")
            nc.sync.dma_start(t[:], d["xq8"][blk])
            x8_sb["q", blk] = t
            t = const.tile([128, 1152], FP8, tag=f"dw8q{blk}", name=f"dw8q{blk}")
            nc.sync.dma_start(t[:], d["dwq8"][blk])
            dwd8_sb["q", blk] = t
        for blk in range(2):
            t = const.tile([128, PAD], FP8, tag=f"x8kv{blk}", name=f"x8kv{blk}")
            nc.sync.dma_start(t[:], d["xkv8"][blk])
            x8_sb["kv", blk] = t
            t = const.tile([128, 1152], FP8, tag=f"dw8k{blk}", name=f"dw8k{blk}")
            nc.sync.dma_start(t[:], d["dwk8"][blk])
            dwd8_sb["k", blk] = t
        # small consts for the v-path diag construction
        eye_sb = const.tile([128, 128], BF16, tag="eye", name="eye_sb")
        nc.sync.dma_start(eye_sb[:], d["eye"])
        dwt_sb = const.tile([128, 54], BF16, tag="dwt", name="dwt_sb")
        nc.sync.dma_start(dwt_sb[:], d["dwt"])
        x_sb = {}
        for blk in range(2):
            t = const.tile([128, PAD], BF16, tag=f"xkv{blk}", name=f"xkv{blk}")
            nc.sync.dma_start(t[:], d["xkv"][blk])
            x_sb["kv", blk] = t
        # weights needed from ~10us on
        w_sb = {p: [const.tile([128, 512], BF16, tag=f"w{p}{kc}", name=f"w{p}{kc}")
                    for kc in range(2)] for p in ("q", "k", "v")}
        for p in ("q", "k", "v"):
            for kc in range(2):
                nc.sync.dma_start(w_sb[p][kc][:], wmap[p][kc])
        qkb_sb = const.tile([128, 8], F32, tag="qkb", name="qkb")
        nc.sync.dma_start(qkb_sb[:], d["qk_bias"])
        ones_bc_sb = const.tile([1, 64], BF16, tag="onesbc", name="ones_bc_sb")
        nc.sync.dma_start(ones_bc_sb[:], d["ones_bc"])
        ffnw_sb = [const.tile([128, 256], BF16, tag=f"ffnw{h}", name=f"ffnw{h}")
                   for h in range(4)]
        for h in range(4):
            nc.sync.dma_start(ffnw_sb[h][:], d["ffnw"][h])
        ffnb_sb = const.tile([128, 256], F32, tag="ffnb", name="ffnbsb")
        nc.sync.dma_start(ffnb_sb[:], d["ffnb"])

        # persistent activations (bf16)
        y_sb = {}   # dw conv outputs, filled in phase order
        q_sb = [const.tile([128, HWN], BF16, tag=f"qsb{i}", name=f"qsb{i}") for i in range(4)]
        k_sb = [const.tile([128, HWN], BF16, tag=f"ksb{i}", name=f"ksb{i}") for i in range(4)]
        vt_sb = [const.tile([128, 8 * 66], BF16, tag=f"vt{i}", name=f"vt{i}") for i in range(8)]
        ot_sb = [const.tile([128, HWN], BF16, tag=f"ot{i}", name=f"ot{i}") for i in range(4)]
        for i in range(8):
            vtv = vt_sb[i][:].rearrange("p (h c) -> p h c", c=66)
            nc.sync.dma_start(vtv[:, :, 64:65], d["vt_ones"])

        # dw diag construction on DVE (v path only; q,k ship fp8 from host)
        dwd_sb = {}
        for ci, p in enumerate(("v",)):
            ci = 2
            for blk in range(2):
                t = const.tile([128, 9 * 128], BF16, tag=f"dw{p}{blk}", name=f"dwd{p}{blk}")
                i0 = ci * 18 + blk * 9
                e3 = eye_sb[:].rearrange("p (a c) -> p a c", a=1)
                w3 = dwt_sb[:, i0:i0 + 9].rearrange("p (a c) -> p a c", c=1)
                e3b, w3b = bass.broadcast_tensor_aps(e3, w3)
                nc.vector.tensor_tensor(
                    t[:].rearrange("p (a c) -> p a c", c=128), e3b, w3b,
                    op=mybir.AluOpType.mult)
                dwd_sb[p, blk] = t

        def emit_dw(p, xin, blk, pool):
            """depthwise conv for proj p, k-block blk -> y_sb[p, blk] bf16"""
            ps = pool.tile([128, HWN], F32, tag="ps", name=f"psdw{p}{blk}")
            xv = x_sb[xin, blk][:].rearrange("p (r c) -> p r c", c=34)
            for tap in range(9):
                di, dj = tap // 3, tap % 3
                lhsT = dwd_sb[p, blk][:, tap * 128:(tap + 1) * 128]
                mm(ps[:], lhsT, xv[:, di: di + 32, dj: dj + 32],
                   start=(tap == 0), stop=(tap == 8))
            y = const.tile([128, HWN], BF16, tag=f"y{p}{blk}", name=f"y{p}{blk}")
            nc.vector.tensor_copy(y[:], ps[:])
            y_sb[p, blk] = y

        def emit_pw_qk(pair, pool):
            """pointwise conv for q and k, oc block = pair (2 heads)."""
            for p, dest in (("q", q_sb), ("k", k_sb)):
                ps = pool.tile([128, HWN], F32, tag="ps", name=f"pspw{p}{pair}")
                for kc in range(2):
                    mm(ps[:], w_sb[p][kc][:, pair * 128:(pair + 1) * 128],
                       y_sb[p, kc][:], start=(kc == 0), stop=(kc == 1))
                ci = 0 if p == "q" else 1
                nc.vector.tensor_scalar_add(
                    dest[pair][:], ps[:], qkb_sb[:, ci * 4 + pair: ci * 4 + pair + 1])

        def emit_pw_v(half, pool):
            """pointwise conv for v, hw blocks [2*half, 2*half+1], transposed."""
            ps = pool.tile([128, HWN], F32, tag="ps", name=f"psv{half}")
            for sub in range(2):
                mb = 2 * half + sub
                for kc in range(2):
                    mm(ps[:, sub * 512:(sub + 1) * 512],
                       y_sb["v", kc][:, mb * 128:(mb + 1) * 128],
                       w_sb["v"][kc][:], start=(kc == 0), stop=(kc == 1))
            for sub in range(2):
                mb = 2 * half + sub
                vtv = vt_sb[mb][:].rearrange("p (h c) -> p h c", c=66)
                nc.vector.tensor_copy(vtv[:, :, 0:64],
                                      ps[:, sub * 512:(sub + 1) * 512])

        # ---------------- fused conv + attention pipeline ----------------
        # ACT-engine EXP (64 x 1.15us) is the serial floor; scores for head
        # order[i+2] are jb-interleaved with av for head order[i] so the PE
        # tracks ACT pacing, and the v-path convs run as fillers inside the
        # EXP-paced h0/h1 scores stretch (their PSUM pool closes before the
        # attention accumulators open).
        from contextlib import ExitStack
        order = [0, 1, 2, 3, 4, 5, 7, 6]
        with tc.tile_pool(name="epool", bufs=24) as epool, \
             tc.tile_pool(name="ps", bufs=2, space="PSUM") as pspool, \
             tc.tile_pool(name="norm", bufs=2) as npool:
            e_tiles = {}

            def emit_scores_jb(h, jb):
                pair, pb = h // 2, (h % 2) * 64
                sp = pspool.tile([128, HWN], F32, tag="ps", name="sp")
                for hf in range(2):
                    mm(sp[:, hf * 512:(hf + 1) * 512],
                       k_sb[pair][pb:pb + 64, jb * 128:(jb + 1) * 128],
                       q_sb[pair][pb:pb + 64, hf * 512:(hf + 1) * 512],
                       start=True, stop=True)
                e = epool.tile([128, HWN], BF16, tag="e", name="e")
                nc.scalar.activation(e[:], sp[:],
                                     mybir.ActivationFunctionType.Exp,
                                     scale=SCALE / 256.0)
                e_tiles[h, jb] = e

            def emit_av_jb(h, jb, oacc):
                for hf in range(2):
                    mm(oacc[:, hf * 512:(hf + 1) * 512],
                       vt_sb[jb][:, 66 * h: 66 * h + 65],
                       e_tiles[h, jb][:, hf * 512:(hf + 1) * 512],
                       start=(jb == 0), stop=(jb == 7))

            def emit_norm(h, oacc, opool, tag):
                pair, pb = h // 2, (h % 2) * 64
                o_un = npool.tile([65, HWN], F32, tag="oun", name="o_un")
                nc.vector.tensor_copy(o_un[:], oacc[:])
                # reciprocal of the denominator row: reshape to [64,16] via DMA
                # (a [1,1024] single-lane DVE reciprocal measures 6.5us)
                csp = npool.tile([64, 16], F32, tag="csp", name="csp")
                nc.sync.dma_start(
                    csp[:], o_un[64:65, :].rearrange("p (a b) -> p a b", b=16))
                csr = npool.tile([64, 16], BF16, tag="csr", name="csr")
                nc.vector.reciprocal(csr[:], csp[:])
                rrow = npool.tile([1, HWN], BF16, tag="rrow", name="rrow")
                nc.sync.dma_start(
                    rrow[:].rearrange("p (a b) -> p a b", b=16), csr[:])
                # rank-1 broadcast of rrow across 64 partitions (reuses oacc bank)
                bc = opool.tile([64, HWN], F32, tag=tag, name=f"bc{h}")
                for hf in range(2):
                    mm(bc[:, hf * 512:(hf + 1) * 512], ones_bc_sb[0:1, :],
                       rrow[0:1, hf * 512:(hf + 1) * 512], start=True, stop=True)
                if pb == 0:
                    nc.vector.tensor_mul(ot_sb[pair][0:64, :], o_un[0:64, :], bc[:])
                else:
                    # DVE lanes can't cross partitions: normalize at base 0,
                    # then DMA the finished bf16 slice up to partitions 64:128
                    tmp = npool.tile([64, HWN], BF16, tag="otmp", name="ot_tmp")
                    nc.vector.tensor_mul(tmp[:], o_un[0:64, :], bc[:])
                    nc.sync.dma_start(ot_sb[pair][64:128, :], tmp[:])

            def gen_dw_dr(p, blk, pool):
                """fp8 DoubleRow depthwise conv: 2 taps contracted per matmul
                via overlapping-window access patterns."""
                import bass_rust
                ps = pool.tile([128, HWN], F32, tag="ps", name=f"ps8{p}{blk}")
                xv = x8_sb["q" if p == "q" else "kv", blk][:].rearrange(
                    "p (r c) -> p r c", c=34)
                for pi, (t1, t2) in enumerate(DR_PAIRS):
                    d1, j1 = t1 // 3, t1 % 3
                    d2, j2 = t2 // 3, t2 % 3
                    delta = (d2 - d1) * 34 + (j2 - j1)
                    lhsT = dwd8_sb[p, blk][:, 256 * pi: 256 * pi + 256].rearrange(
                        "p (a c) -> p a c", c=128)
                    for hf in range(2):
                        rhs = xv[:, d1 + hf * 16: d1 + hf * 16 + 16,
                                 j1: j1 + 32].unsqueeze(1).broadcast_to(
                                     [128, 2, 16, 32])
                        rhs.ap = bass_rust.VecI64Pair(
                            [[PAD, 128], [delta, 2], [34, 16], [1, 32]])
                        mm(ps[:, hf * 512:(hf + 1) * 512], lhsT, rhs,
                           start=(pi == 0), stop=False,
                           perf_mode=mybir.MatmulPerfMode.DoubleRow)
                    yield
                lhsT8 = dwd8_sb[p, blk][:, 1024:1152]
                for hf in range(2):
                    rhs = xv[:, 2 + hf * 16: 2 + hf * 16 + 16, 2:34]
                    mm(ps[:, hf * 512:(hf + 1) * 512], lhsT8, rhs,
                       start=False, stop=True)
                yield
                y = const.tile([128, HWN], BF16, tag=f"y{p}{blk}", name=f"y{p}{blk}")
                nc.vector.tensor_copy(y[:], ps[:])
                y_sb[p, blk] = y

            def gen_dw(p, xin, blk, pool):
                ps = pool.tile([128, HWN], F32, tag="ps", name=f"psdw{p}{blk}")
                xv = x_sb[xin, blk][:].rearrange("p (r c) -> p r c", c=34)
                for tap in range(9):
                    di, dj = tap // 3, tap % 3
                    lhsT = dwd_sb[p, blk][:, tap * 128:(tap + 1) * 128]
                    for hf in range(2):
                        rhs = xv[:, di + hf * 16: di + hf * 16 + 16, dj: dj + 32]
                        mm(ps[:, hf * 512:(hf + 1) * 512], lhsT, rhs,
                           start=(tap == 0), stop=(tap == 8))
                    yield
                y = const.tile([128, HWN], BF16, tag=f"y{p}{blk}", name=f"y{p}{blk}")
                nc.vector.tensor_copy(y[:], ps[:])
                y_sb[p, blk] = y

            def gen_pw_qk(pair, pool):
                for p, dest in (("q", q_sb), ("k", k_sb)):
                    ps = pool.tile([128, HWN], F32, tag="ps", name=f"pspw{p}{pair}")
                    for kc in range(2):
                        for hf in range(2):
                            mm(ps[:, hf * 512:(hf + 1) * 512],
                               w_sb[p][kc][:, pair * 128:(pair + 1) * 128],
                               y_sb[p, kc][:, hf * 512:(hf + 1) * 512],
                               start=(kc == 0), stop=(kc == 1))
                        yield
                    ci = 0 if p == "q" else 1
                    nc.vector.tensor_scalar_add(
                        dest[pair][:], ps[:],
                        qkb_sb[:, ci * 4 + pair: ci * 4 + pair + 1])

            def gen_pw_v(half, pool):
                ps = pool.tile([128, HWN], F32, tag="ps", name=f"psv{half}")
                for sub in range(2):
                    mb = 2 * half + sub
                    for kc in range(2):
                        mm(ps[:, sub * 512:(sub + 1) * 512],
                           y_sb["v", kc][:, mb * 128:(mb + 1) * 128],
                           w_sb["v"][kc][:], start=(kc == 0), stop=(kc == 1))
                    yield
                for sub in range(2):
                    mb = 2 * half + sub
                    vtv = vt_sb[mb][:].rearrange("p (h c) -> p h c", c=66)
                    nc.vector.tensor_copy(vtv[:, :, 0:64],
                                          ps[:, sub * 512:(sub + 1) * 512])

            # --- conv stage ---
            with tc.tile_pool(name="psdw", bufs=2, space="PSUM") as psdw:
                for blk in range(2):
                    for _ in gen_dw_dr("q", blk, psdw):
                        pass
                for blk in range(2):
                    for _ in gen_dw_dr("k", blk, psdw):
                        pass
                for _ in gen_pw_qk(0, psdw):
                    pass
                # v path + remaining qk pointwise, stepped as fillers between
                # the EXP-paced h0/h1 scores
                pending = [gen_dw("v", "kv", 0, psdw), gen_pw_qk(1, psdw),
                           gen_dw("v", "kv", 1, psdw), gen_pw_qk(2, psdw),
                           gen_pw_qk(3, psdw)] + \
                          [gen_pw_v(half, psdw) for half in range(4)]

                def step_fill(n):
                    while n > 0 and pending:
                        try:
                            next(pending[0])
                            n -= 1
                        except StopIteration:
                            pending.pop(0)

                for h in order[:2]:
                    for jb in range(8):
                        emit_scores_jb(h, jb)
                        step_fill(2)
                step_fill(10 ** 6)

            # --- attention stage ---
            stA = ExitStack()
            with tc.tile_pool(name="oaccB", bufs=1, space="PSUM") as opB:
                opA = stA.enter_context(
                    tc.tile_pool(name="oaccA", bufs=1, space="PSUM"))
                op = {0: (opA, "oA"), 1: (opB, "oB")}
                for idx in range(6):           # heads 0,1,2,3,4,5
                    h = order[idx]
                    opool, tag = op[idx % 2]
                    oacc = opool.tile([65, HWN], F32, tag=tag, name=f"oacc{h}")
                    h2 = order[idx + 2]
                    for jb in range(8):
                        emit_scores_jb(h2, jb)
                        emit_av_jb(h, jb, oacc)
                    emit_norm(h, oacc, opool, tag)
                # tail: run both remaining av blocks back to back so the
                # normalize chains (DVE/DMA latency) hide under PE work
                oacc7 = opA.tile([65, HWN], F32, tag="oA", name="oacc7")
                for jb in range(8):
                    emit_av_jb(7, jb, oacc7)
                oacc6 = opB.tile([65, HWN], F32, tag="oB", name="oacc6")
                for jb in range(8):
                    emit_av_jb(6, jb, oacc6)
                emit_norm(7, oacc7, opA, "oA")
                stA.close()   # frees 2 banks for the early ffn blocks
                with tc.tile_pool(name="psf", bufs=1, space="PSUM") as psf:
                    # one open accumulation group per 2KB PSUM bank
                    f_t = [psf.tile([128, 512], F32, tag=f"f{i}", name=f"psf{i}")
                           for i in range(2)]
                    for nb in range(2):        # nb 0..1: kc 0..2 pre-issued
                        for kc in range(3):
                            mm(f_t[nb][:, 0:256],
                               ot_sb[kc][:, nb * 128:(nb + 1) * 128],
                               ffnw_sb[kc][:], start=(kc == 0), stop=False)
                    emit_norm(6, oacc6, opB, "oB")
                    with tc.tile_pool(name="fout", bufs=4) as fpool:
                        def fin(nb):
                            fo = fpool.tile([128, 256], F32, tag="fo", name="fo")
                            nc.vector.tensor_tensor(
                                fo[:], f_t[nb % 2][:, 0:256],
                                ffnb_sb[:], op=mybir.AluOpType.add)
                            nc.sync.dma_start(
                                d["out"][nb * 128:(nb + 1) * 128, :], fo[:])
                        for nb in range(2):
                            mm(f_t[nb][:, 0:256],
                               ot_sb[3][:, nb * 128:(nb + 1) * 128],
                               ffnw_sb[3][:], start=False, stop=True)
                            fin(nb)
                        for nb in range(2, 8):
                            for kc in range(4):
                                mm(f_t[nb % 2][:, 0:256],
                                   ot_sb[kc][:, nb * 128:(nb + 1) * 128],
                                   ffnw_sb[kc][:], start=(kc == 0), stop=(kc == 3))
                            fin(nb)


def _build():
    nc = bacc.Bacc("TRN2", target_bir_lowering=False, debug=False)
    with tile.TileContext(nc) as tc:
        _emit(nc, tc)
    nc.compile()
    return nc


# ----------------------------------------------------------------- host code

def _host_shared(inputs):
    g = lambda n: np.asarray(inputs[n], dtype=np.float32)
    d = {}
    dw_effs = []
    qk_bias_cols = []
    for ci, p in enumerate(("q", "k", "v")):
        a = g(f"{p}_bn_g") / np.sqrt(g(f"{p}_bn_v") + EPS)          # [256]
        dw_eff = g(f"{p}_dw_w")[:, 0] * a[:, None, None]            # [256,3,3]
        beta = a * g(f"{p}_dw_b") + g(f"{p}_bn_b") - a * g(f"{p}_bn_m")
        pw = g(f"{p}_pw_w")[:, :, 0, 0]                             # [512,256]
        bias = g(f"{p}_pw_b") + pw @ beta                           # [512]
        dw_effs.append(dw_eff)
        d[f"w{p}"] = np.ascontiguousarray(pw.T.reshape(2, 128, 512)).astype(np.float32)
        if p == "v":
            vbias = bias                                            # [512]
        else:
            qk_bias_cols.append(bias)
    # q,k depthwise weights ship as fp8 scaled x16 (folded back out of the
    # softmax via EXP scale /256); biases carry the same x16
    qkb = np.zeros((128, 8), np.float32)
    for ci in range(2):
        for mb in range(4):
            qkb[:, ci * 4 + mb] = 16.0 * qk_bias_cols[ci][mb * 128:(mb + 1) * 128]
    d["qk_bias"] = qkb
    import ml_dtypes
    DR_PAIRS = [(0, 1), (3, 4), (6, 7), (2, 5)]
    for nm, ci in (("dwq8", 0), ("dwk8", 1)):
        w = dw_effs[ci].reshape(256, 9) * 16.0          # [256, 9]
        m = np.zeros((2, 128, 1152), np.float32)
        for blk in range(2):
            wb = w[blk * 128:(blk + 1) * 128]
            for pi, (t1, t2) in enumerate(DR_PAIRS):
                m[blk, :, 256 * pi: 256 * pi + 128] = np.diag(wb[:, t1])
                m[blk, :, 256 * pi + 128: 256 * pi + 256] = np.diag(wb[:, t2])
            m[blk, :, 1024:1152] = np.diag(wb[:, 8])
        d[nm] = m.astype(ml_dtypes.float8_e4m3fn)
    d["eye"] = np.eye(128, dtype=np.float32)
    dwtm = np.zeros((128, 54), np.float32)
    for ci in range(3):
        for blk in range(2):
            for t in range(9):
                dwtm[:, ci * 18 + blk * 9 + t] = dw_effs[ci][blk * 128:(blk + 1) * 128, t // 3, t % 3]
    d["dwt"] = dwtm
    d["vt_ones"] = np.ones((128, 8, 1), np.float32)
    d["ones_bc"] = np.ones((1, 64), np.float32)
    d["ffnw"] = np.ascontiguousarray(g("ffn_w").T.reshape(4, 128, 256)).astype(np.float32)
    # v pointwise bias folded through the ffn (exact): out += (ffn_w @ v_bias)
    ffnb_eff = g("ffn_b") + g("ffn_w") @ vbias                      # [256]
    d["ffnb"] = np.broadcast_to(ffnb_eff, (128, 256)).copy()
    # bf16 conversion for the matmul operands
    for k in ("wq", "wk", "wv", "eye", "dwt", "vt_ones", "ones_bc", "ffnw"):
        d[k] = d[k].astype(ml_dtypes.bfloat16)
    return d


def _host_x(feat):
    # [1024, 256] -> padded transposed [2, 128, 34*34]
    xt = np.ascontiguousarray(np.asarray(feat, np.float32).T).reshape(2, 128, 32, 32)
    xp = np.zeros((2, 128, 34, 34), np.float32)
    xp[:, :, 1:33, 1:33] = xt
    return xp.reshape(2, 128, PAD)


def make_in_maps(inputs):
    import ml_dtypes
    shared = _host_shared(inputs)
    f1 = np.asarray(inputs["features1"], dtype=np.float32)
    f2 = np.asarray(inputs["features2"], dtype=np.float32)
    maps = []
    for b in range(B):
        m = dict(shared)
        x1 = _host_x(f1[b])
        x2 = _host_x(f2[b])
        m["xq8"] = x1.astype(ml_dtypes.float8_e4m3fn)
        m["xkv8"] = x2.astype(ml_dtypes.float8_e4m3fn)
        m["xkv"] = x2.astype(ml_dtypes.bfloat16)
        maps.append(m)
    return maps


def get_nc():
    if "nc" not in _CACHE:
        _CACHE["nc"] = _build()
    return _CACHE["nc"]


def kernel(**inputs):
    nc = get_nc()
    in_maps = make_in_maps(inputs)
    res = run_bass_kernel_spmd(nc, in_maps, list(range(B)))
    return np.stack([res.results[i]["out"] for i in range(B)]).astype(np.float32)
